# revision 4
# baseline (speedup 1.0000x reference)
"""nn_MiniEmbedding Trainium2 kernel (8 NeuronCores, Bass/Tile).

kernel(**inputs) takes FULL inputs (windows [1024,128,3] + MLP params) and
returns the FULL output [1024,128,128] f32.

Per core (shard of the M axis, pure data parallel):
  KNN top-16 by squared distance: PE gram matmul (key = dot - sq_j/2),
  DVE max8 / match_replace / max_index for top-16 values+indices,
  scale = dist to 16th neighbor, q-space gather (gpsimd ap_gather),
  A1 = (q_idx - q_k)/s, 3-layer MLP on PE (bf16 L2/L3, fp32 accum),
  segmented reduce-max over the 16 neighbors, PE transpose (+b3 preload),
  DMA out.

The compiled program processes NWP windows per core per launch; the driver
loops launches to cover the 128-window shard. Compilation is cached at module
level (first call compiles, subsequent calls only execute).
"""

from contextlib import ExitStack

import numpy as np

M, KPTS, CH = 1024, 128, 3
N_CORES = 8
M_SHARD = M // N_CORES

K = 128          # points per window
L = 16           # neighbors kept
BW = 4           # windows per block
import os as _os
NWP = int(_os.environ.get("NWP", "128"))  # windows per core per launch
BARRIER_EVERY = int(_os.environ.get("BARRIER_EVERY", "1"))
NEG_BIG = -1e30


def _build(ctx, tc, outs, ins, nw):
    import concourse.mybir as mybir

    FP32 = mybir.dt.float32
    BF16 = mybir.dt.bfloat16
    U16 = mybir.dt.uint16
    I16 = mybir.dt.int16
    AF = mybir.ActivationFunctionType

    nc = tc.nc
    assert nw % BW == 0
    nblk = nw // BW

    winP = ins["windowsP"]
    winQ = ins["windowsQ"]
    out_d = outs["out"]

    cpool = ctx.enter_context(tc.tile_pool(name="consts", bufs=1))
    chpool = ctx.enter_context(tc.tile_pool(name="chunks", bufs=3))
    kpool = ctx.enter_context(tc.tile_pool(name="keys", bufs=2))
    qpool = ctx.enter_context(tc.tile_pool(name="q", bufs=2))
    selpool = ctx.enter_context(tc.tile_pool(name="sel", bufs=2))
    gpool = ctx.enter_context(tc.tile_pool(name="gath", bufs=2))
    hpool = ctx.enter_context(tc.tile_pool(name="acts", bufs=2))
    opool = ctx.enter_context(tc.tile_pool(name="outs", bufs=2))
    keys_pp = ctx.enter_context(tc.tile_pool(name="keys_ps", bufs=1, space="PSUM"))
    mini_pp = ctx.enter_context(tc.tile_pool(name="mini_ps", bufs=1, space="PSUM"))
    a2_pp = ctx.enter_context(tc.tile_pool(name="a2_ps", bufs=1, space="PSUM"))
    a3_pp = ctx.enter_context(tc.tile_pool(name="a3_ps", bufs=2, space="PSUM"))

    w1_sb = cpool.tile([128, 32], FP32)
    nc.sync.dma_start(w1_sb[:], ins["w1"])
    w2_sb = cpool.tile([128, 64], BF16)
    nc.sync.dma_start(w2_sb[:], ins["w2b"])
    w3_sb = cpool.tile([128, 128], BF16)
    nc.sync.dma_start(w3_sb[:], ins["w3b"])
    b1col = cpool.tile([128, 1], FP32)
    nc.sync.dma_start(b1col[:], ins["b1col"])
    b2col = cpool.tile([128, 1], FP32)
    nc.sync.dma_start(b2col[:], ins["b2col"])
    b3row = cpool.tile([1, 128], FP32)
    nc.sync.dma_start(b3row[:], ins["b3row"])
    rep4 = cpool.tile([4, 128], FP32)
    nc.sync.dma_start(rep4[:], ins["rep4"])
    onesrow = cpool.tile([1, 128], FP32)
    nc.sync.dma_start(onesrow[:], ins["onesrow"])
    ident = cpool.tile([128, 128], FP32)
    nc.sync.dma_start(ident[:], ins["ident"])
    sq_all = cpool.tile([128, nw], FP32)
    nc.sync.dma_start(sq_all[:], ins["sqT"])

    for b in range(nblk):
        wins = [BW * b + a for a in range(BW)]
        if b > 0 and b % BARRIER_EVERY == 0:
            tc.strict_bb_all_engine_barrier()

        chA = chpool.tile([128, 128], FP32, tag="chA")
        chB = chpool.tile([128, 128], FP32, tag="chB")
        for a, w in enumerate(wins):
            nc.sync.dma_start(chA[32 * a:32 * a + 4, :], winP[w])
            nc.sync.dma_start(chB[32 * a:32 * a + 4, :], winQ[w])

        # gram: key = [x,y,z,1].T @ [x,y,z,-sq/2] = dot - 0.5*sq_j
        keys_ps = keys_pp.tile([128, BW * 128], FP32)
        for a in range(BW):
            nc.tensor.matmul(
                keys_ps[:, 128 * a:128 * (a + 1)], chA[32 * a:32 * a + 4, :],
                chB[32 * a:32 * a + 4, :], start=True, stop=True,
                tile_position=(32 * a, 0))
        keys_sb = kpool.tile([128, BW * 128], FP32)
        nc.scalar.activation(keys_sb[:], keys_ps[:], AF.Copy)

        # Q = W1.T @ pts per window -> [128 (4w x 32ch), 128 pts]
        q4_sb = qpool.tile([128, 128], FP32)
        for a in range(BW):
            q1_ps = mini_pp.tile([32, 128], FP32, tag="mini")
            nc.tensor.matmul(q1_ps[:], w1_sb[32 * a:32 * a + 3, :],
                             chA[32 * a:32 * a + 3, :], start=True, stop=True,
                             tile_position=(32 * a, 0))
            nc.scalar.activation(q4_sb[32 * a:32 * (a + 1), :], q1_ps[:],
                                 AF.Copy)

        # top-16 selection per row; indices doubled into a layout whose
        # transpose is the ap_gather wrapped-index layout.
        v16 = selpool.tile([128, BW * 16], FP32, tag="v16")
        i16pad = selpool.tile([128, 128], U16, tag="i16")
        keyrep = selpool.tile([128, BW * 128], FP32, tag="keyrep")
        for a in range(BW):
            key = keys_sb[:, 128 * a:128 * (a + 1)]
            key2 = keyrep[:, 128 * a:128 * (a + 1)]
            va = v16[:, 16 * a:16 * a + 8]
            vb = v16[:, 16 * a + 8:16 * a + 16]
            nc.vector.max(va, key)
            nc.vector.max_index(i16pad[:, 32 * a:32 * a + 8], va, key)
            nc.vector.match_replace(key2, va, key, NEG_BIG)
            nc.vector.max(vb, key2)
            nc.vector.max_index(i16pad[:, 32 * a + 8:32 * a + 16], vb, key2)
        nc.vector.tensor_copy(
            i16pad[:].rearrange("p (w r l) -> p w r l", r=2, l=16)[:, :, 1, :],
            i16pad[:].rearrange("p (w r l) -> p w r l", r=2, l=16)[:, :, 0, :])

        # scale: s = sqrt(max(sq_k - 2*key16, 0)); inv = 1/max(s, 1e-8)
        invb = selpool.tile([128, BW], FP32, tag="invb")
        v16min = v16[:].rearrange("p (w l) -> p w l", l=16)[:, :, 15]
        s2 = selpool.tile([128, BW], FP32, tag="s2")
        nc.vector.tensor_scalar_mul(s2[:], v16min, -2.0)
        nc.vector.tensor_add(s2[:], s2[:], sq_all[:, BW * b:BW * (b + 1)])
        nc.vector.tensor_scalar_max(s2[:], s2[:], 0.0)
        s1 = selpool.tile([128, BW], FP32, tag="s1")
        nc.scalar.activation(s1[:], s2[:], AF.Sqrt)
        nc.vector.tensor_scalar_max(s1[:], s1[:], 1e-8)
        nc.vector.reciprocal(invb[:], s1[:])

        # inv_s4[p, k] = invb[k, p//32] via transpose-matmul + replication
        invT_ps = mini_pp.tile([BW, 128], FP32, tag="mini")
        nc.tensor.matmul(invT_ps[:], invb[:], ident[:], start=True, stop=True)
        invT_sb = selpool.tile([BW, 128], FP32, tag="invT")
        nc.scalar.activation(invT_sb[:], invT_ps[:], AF.Copy)
        invs4_ps = mini_pp.tile([128, 128], FP32, tag="mini")
        nc.tensor.matmul(invs4_ps[:], rep4[:], invT_sb[:], start=True,
                         stop=True)
        invs4_sb = selpool.tile([128, 128], FP32, tag="invs4")
        nc.scalar.activation(invs4_sb[:], invs4_ps[:], AF.Copy)

        # index transpose via PE (indices as exact fp32)
        idx128 = selpool.tile([128, 128], I16, tag="idx128")
        idxf = selpool.tile([128, 128], FP32, tag="idxf")
        nc.vector.tensor_copy(idxf[:], i16pad[:])
        idxt_ps = mini_pp.tile([128, 128], FP32, tag="mini")
        nc.tensor.matmul(idxt_ps[:], idxf[:], ident[:], start=True, stop=True)
        nc.scalar.activation(idx128[:], idxt_ps[:], AF.Copy)

        # gather Qg[p, k*16+l] = Q4[p, idx[k,l]]
        qg = gpool.tile([128, K * L], FP32, tag="qg")
        nc.gpsimd.ap_gather(
            qg[:].unsqueeze(2), q4_sb[:].unsqueeze(2), idx128[:],
            channels=128, num_elems=K, d=1, num_idxs=K * L)

        # A1 = (Qg - Q4_rep) * inv_rep ; h1 = relu(A1 + b1) -> bf16
        q4rep = q4_sb[:].unsqueeze(2).broadcast_to([128, K, L])
        invrep = invs4_sb[:].unsqueeze(2).broadcast_to([128, K, L])
        a1 = gpool.tile([128, K, L], FP32, tag="a1")
        nc.vector.tensor_sub(a1[:], qg[:].rearrange("p (k l) -> p k l", l=L),
                             q4rep)
        nc.vector.tensor_mul(a1[:], a1[:], invrep)
        h1b = hpool.tile([128, K * L], BF16, tag="h1b")
        nc.scalar.activation(h1b[:].rearrange("p (k l) -> p k l", l=L), a1[:],
                             AF.Relu, bias=b1col[:])

        # L2 + relu2 + L3 + reduce over neighbors
        m3 = opool.tile([128, BW * 128], FP32, tag="m3")
        for p in range(BW // 2):
            a2_ps = a2_pp.tile([128, 2048], FP32)
            for i in range(2):
                a = 2 * p + i
                for h in range(4):
                    nc.tensor.matmul(
                        a2_ps[64 * i:64 * (i + 1), 512 * h:512 * (h + 1)],
                        w2_sb[32 * a:32 * (a + 1), :],
                        h1b[32 * a:32 * (a + 1), 512 * h:512 * (h + 1)],
                        start=True, stop=True,
                        tile_position=(32 * a, 64 * i))
            h2b = hpool.tile([128, 2048], BF16, tag="h2b")
            nc.scalar.activation(h2b[:], a2_ps[:], AF.Relu, bias=b2col[:])
            for i in range(2):
                a = 2 * p + i
                for h in range(4):
                    a3_ps = a3_pp.tile([128, 512], FP32)
                    nc.tensor.matmul(
                        a3_ps[:], w3_sb[64 * i:64 * (i + 1), :],
                        h2b[64 * i:64 * (i + 1), 512 * h:512 * (h + 1)],
                        start=True, stop=True, tile_position=(64 * i, 0))
                    nc.vector.reduce_max(
                        m3[:, 128 * a + 32 * h:128 * a + 32 * (h + 1)],
                        a3_ps[:].rearrange("p (k l) -> p k l", l=L),
                        axis=mybir.AxisListType.X)

        # b3 preload + PE transpose + copy out + DMA
        for a, w in enumerate(wins):
            tr_ps = mini_pp.tile([128, 128], FP32, tag="mini")
            nc.tensor.matmul(tr_ps[:], onesrow[:], b3row[:],
                             start=True, stop=False)
            nc.tensor.matmul(tr_ps[:], m3[:, 128 * a:128 * (a + 1)], ident[:],
                             is_transpose=True, start=False, stop=True)
            out_sb = opool.tile([128, 128], FP32, tag="osb")
            nc.scalar.activation(out_sb[:], tr_ps[:], AF.Copy)
            nc.sync.dma_start(out_d[w], out_sb[:])


_CACHE = {}


def _get_program():
    if "nc" in _CACHE:
        return _CACHE["nc"]
    import concourse.mybir as mybir
    import concourse.tile as tile
    from concourse import bacc

    nc = bacc.Bacc("TRN2", target_bir_lowering=False, debug=False,
                   num_devices=N_CORES)
    specs = {
        "windowsP": ((NWP, 4, 128), mybir.dt.float32),
        "windowsQ": ((NWP, 4, 128), mybir.dt.float32),
        "sqT": ((128, NWP), mybir.dt.float32),
        "w1": ((128, 32), mybir.dt.float32),
        "w2b": ((128, 64), mybir.dt.bfloat16),
        "w3b": ((128, 128), mybir.dt.bfloat16),
        "b1col": ((128, 1), mybir.dt.float32),
        "b2col": ((128, 1), mybir.dt.float32),
        "b3row": ((1, 128), mybir.dt.float32),
        "rep4": ((4, 128), mybir.dt.float32),
        "onesrow": ((1, 128), mybir.dt.float32),
        "ident": ((128, 128), mybir.dt.float32),
    }
    ins = {k: nc.dram_tensor(k, list(shape), dt, kind="ExternalInput").ap()
           for k, (shape, dt) in specs.items()}
    outs = {"out": nc.dram_tensor("out", [NWP, 128, 128], mybir.dt.float32,
                                  kind="ExternalOutput").ap()}
    with tile.TileContext(nc) as tc:
        ctx = ExitStack()
        with ctx:
            _build(ctx, tc, outs, ins, NWP)
    nc.compile()
    _CACHE["nc"] = nc
    return nc


def _make_consts(W1, b1, W2, b2, W3, b3):
    import ml_dtypes
    w1r = np.zeros((128, 32), np.float32)
    for a in range(4):
        w1r[32 * a:32 * a + 3] = W1
    rep4 = np.zeros((4, 128), np.float32)
    for a in range(4):
        rep4[a, 32 * a:32 * a + 32] = 1.0
    return {
        "w1": w1r,
        "w2b": np.tile(np.asarray(W2, np.float32), (4, 1)).astype(
            ml_dtypes.bfloat16),
        "w3b": np.tile(np.asarray(W3, np.float32), (2, 1)).astype(
            ml_dtypes.bfloat16),
        "b1col": np.tile(np.asarray(b1, np.float32), 4)[:, None].copy(),
        "b2col": np.tile(np.asarray(b2, np.float32), 2)[:, None].copy(),
        "b3row": np.asarray(b3, np.float32)[None, :].copy(),
        "rep4": rep4,
        "onesrow": np.ones((1, 128), np.float32),
        "ident": np.eye(128, dtype=np.float32),
    }


def _prep_windows(pts):
    """pts [nw,128,3] -> windowsP/windowsQ [nw,4,128] + sqT [128,nw]."""
    nw = pts.shape[0]
    sq = np.einsum("wkc,wkc->wk", pts, pts)
    winP = np.concatenate([pts, np.ones((nw, K, 1), np.float32)],
                          axis=2).transpose(0, 2, 1)
    winQ = np.concatenate([pts, -0.5 * sq[:, :, None]],
                          axis=2).transpose(0, 2, 1)
    return (np.ascontiguousarray(winP), np.ascontiguousarray(winQ),
            np.ascontiguousarray(sq.T))


def kernel(windows, W1, b1, W2, b2, W3, b3):
    from concourse.bass_utils import run_bass_kernel_spmd

    windows = np.ascontiguousarray(windows, dtype=np.float32)
    consts = _make_consts(W1, b1, W2, b2, W3, b3)
    nc = _get_program()

    out = np.empty((M, 128, 128), np.float32)
    n_launch = M_SHARD // NWP
    for j in range(n_launch):
        in_maps = []
        for c in range(N_CORES):
            lo = c * M_SHARD + j * NWP
            winP, winQ, sqT = _prep_windows(windows[lo:lo + NWP])
            in_maps.append({"windowsP": winP, "windowsQ": winQ, "sqT": sqT,
                            **consts})
        res = run_bass_kernel_spmd(nc, in_maps,
                                   core_ids=list(range(N_CORES)))
        for c in range(N_CORES):
            lo = c * M_SHARD + j * NWP
            out[lo:lo + NWP] = res.results[c]["out"]
    return out


if __name__ == "__main__":
    rng = np.random.default_rng(0)
    w = rng.standard_normal((M, 128, 3)).astype(np.float32)

    def _lin(fi, fo):
        lim = 1.0 / np.sqrt(fi)
        return (rng.uniform(-lim, lim, (fi, fo)).astype(np.float32),
                rng.uniform(-lim, lim, fo).astype(np.float32))

    W1, b1 = _lin(3, 32)
    W2, b2 = _lin(32, 64)
    W3, b3 = _lin(64, 128)
    o = kernel(w, W1=W1, b1=b1, W2=W2, b2=b2, W3=W3, b3=b3)
    print(o.shape, o.dtype, float(np.abs(o).max()))


# revision 5
# speedup vs baseline: 1.0444x; 1.0444x over previous
"""nn_MiniEmbedding Trainium2 kernel (8 NeuronCores, Bass/Tile).

kernel(**inputs) takes FULL inputs (windows [1024,128,3] + MLP params) and
returns the FULL output [1024,128,128] f32.

Per core (shard of the M axis, pure data parallel):
  KNN top-16 by squared distance: PE gram matmul (key = dot - sq_j/2),
  DVE max8 / match_replace / max_index for top-16 values+indices,
  scale = dist to 16th neighbor, q-space gather (gpsimd ap_gather),
  A1 = (q_idx - q_k)/s, 3-layer MLP on PE (bf16 L2/L3, fp32 accum),
  segmented reduce-max over the 16 neighbors, PE transpose (+b3 preload),
  DMA out.

The compiled program processes NWP windows per core per launch; the driver
loops launches to cover the 128-window shard. Compilation is cached at module
level (first call compiles, subsequent calls only execute).
"""

from contextlib import ExitStack

import numpy as np

M, KPTS, CH = 1024, 128, 3
N_CORES = 8
M_SHARD = M // N_CORES

K = 128          # points per window
L = 16           # neighbors kept
BW = 4           # windows per block
import os as _os
NWP = int(_os.environ.get("NWP", "128"))  # windows per core per launch
BARRIER_EVERY = int(_os.environ.get("BARRIER_EVERY", "1"))
NEG_BIG = -1e30


def _build(ctx, tc, outs, ins, nw):
    import concourse.mybir as mybir

    FP32 = mybir.dt.float32
    BF16 = mybir.dt.bfloat16
    U16 = mybir.dt.uint16
    I16 = mybir.dt.int16
    AF = mybir.ActivationFunctionType

    nc = tc.nc
    assert nw % BW == 0
    nblk = nw // BW

    winP = ins["windowsP"]
    winQ = ins["windowsQ"]
    out_d = outs["out"]

    cpool = ctx.enter_context(tc.tile_pool(name="consts", bufs=1))
    chpool = ctx.enter_context(tc.tile_pool(name="chunks", bufs=3))
    kpool = ctx.enter_context(tc.tile_pool(name="keys", bufs=2))
    qpool = ctx.enter_context(tc.tile_pool(name="q", bufs=2))
    selpool = ctx.enter_context(tc.tile_pool(name="sel", bufs=2))
    gpool = ctx.enter_context(tc.tile_pool(name="gath", bufs=2))
    hpool = ctx.enter_context(tc.tile_pool(name="acts", bufs=2))
    opool = ctx.enter_context(tc.tile_pool(name="outs", bufs=2))
    keys_pp = ctx.enter_context(tc.tile_pool(name="keys_ps", bufs=1, space="PSUM"))
    mini_pp = ctx.enter_context(tc.tile_pool(name="mini_ps", bufs=1, space="PSUM"))
    a2_pp = ctx.enter_context(tc.tile_pool(name="a2_ps", bufs=1, space="PSUM"))
    a3_pp = ctx.enter_context(tc.tile_pool(name="a3_ps", bufs=2, space="PSUM"))

    w1_sb = cpool.tile([128, 32], FP32)
    nc.sync.dma_start(w1_sb[:], ins["w1"])
    w2_sb = cpool.tile([128, 64], BF16)
    nc.sync.dma_start(w2_sb[:], ins["w2b"])
    w3_sb = cpool.tile([128, 128], BF16)
    nc.sync.dma_start(w3_sb[:], ins["w3b"])
    b1col = cpool.tile([128, 1], FP32)
    nc.sync.dma_start(b1col[:], ins["b1col"])
    b2col = cpool.tile([128, 1], FP32)
    nc.sync.dma_start(b2col[:], ins["b2col"])
    b3row = cpool.tile([1, 128], FP32)
    nc.sync.dma_start(b3row[:], ins["b3row"])
    rep4 = cpool.tile([4, 128], FP32)
    nc.sync.dma_start(rep4[:], ins["rep4"])
    onesrow = cpool.tile([1, 128], FP32)
    nc.sync.dma_start(onesrow[:], ins["onesrow"])
    ident = cpool.tile([128, 128], FP32)
    nc.sync.dma_start(ident[:], ins["ident"])
    sq_all = cpool.tile([128, nw], FP32)
    nc.sync.dma_start(sq_all[:], ins["sqT"])

    for b in range(nblk):
        wins = [BW * b + a for a in range(BW)]
        if b > 0 and b % BARRIER_EVERY == 0:
            tc.strict_bb_all_engine_barrier()

        chA = chpool.tile([128, 128], FP32, tag="chA")
        chB = chpool.tile([128, 128], FP32, tag="chB")
        for a, w in enumerate(wins):
            nc.sync.dma_start(chA[32 * a:32 * a + 4, :], winP[w])
            nc.sync.dma_start(chB[32 * a:32 * a + 4, :], winQ[w])

        # gram: key = [x,y,z,1].T @ [x,y,z,-sq/2] = dot - 0.5*sq_j
        keys_ps = keys_pp.tile([128, BW * 128], FP32)
        for a in range(BW):
            nc.tensor.matmul(
                keys_ps[:, 128 * a:128 * (a + 1)], chA[32 * a:32 * a + 4, :],
                chB[32 * a:32 * a + 4, :], start=True, stop=True,
                tile_position=(32 * a, 0))
        keys_sb = kpool.tile([128, BW * 128], FP32)
        nc.scalar.activation(keys_sb[:], keys_ps[:], AF.Copy)

        # Q = W1.T @ pts per window -> [128 (4w x 32ch), 128 pts]
        q4_sb = qpool.tile([128, 128], FP32)
        for a in range(BW):
            q1_ps = mini_pp.tile([32, 128], FP32, tag="mini")
            nc.tensor.matmul(q1_ps[:], w1_sb[32 * a:32 * a + 3, :],
                             chA[32 * a:32 * a + 3, :], start=True, stop=True,
                             tile_position=(32 * a, 0))
            nc.scalar.activation(q4_sb[32 * a:32 * (a + 1), :], q1_ps[:],
                                 AF.Copy)

        # top-16 selection per row; indices doubled into a layout whose
        # transpose is the ap_gather wrapped-index layout.
        v16 = selpool.tile([128, BW * 16], FP32, tag="v16")
        i16pad = selpool.tile([128, 128], U16, tag="i16")
        keyrep = selpool.tile([128, BW * 128], FP32, tag="keyrep")
        for a in range(BW):
            key = keys_sb[:, 128 * a:128 * (a + 1)]
            key2 = keyrep[:, 128 * a:128 * (a + 1)]
            va = v16[:, 16 * a:16 * a + 8]
            vb = v16[:, 16 * a + 8:16 * a + 16]
            nc.vector.max(va, key)
            nc.vector.max_index(i16pad[:, 32 * a:32 * a + 8], va, key)
            nc.vector.match_replace(key2, va, key, NEG_BIG)
            nc.vector.max(vb, key2)
            nc.vector.max_index(i16pad[:, 32 * a + 8:32 * a + 16], vb, key2)
        nc.vector.tensor_copy(
            i16pad[:].rearrange("p (w r l) -> p w r l", r=2, l=16)[:, :, 1, :],
            i16pad[:].rearrange("p (w r l) -> p w r l", r=2, l=16)[:, :, 0, :])

        # scale: s = sqrt(max(sq_k - 2*key16, 0)); inv = 1/max(s, 1e-8)
        invb = selpool.tile([128, BW], FP32, tag="invb")
        v16min = v16[:].rearrange("p (w l) -> p w l", l=16)[:, :, 15]
        s2 = selpool.tile([128, BW], FP32, tag="s2")
        nc.vector.tensor_scalar_mul(s2[:], v16min, -2.0)
        nc.vector.tensor_add(s2[:], s2[:], sq_all[:, BW * b:BW * (b + 1)])
        nc.vector.tensor_scalar_max(s2[:], s2[:], 0.0)
        s1 = selpool.tile([128, BW], FP32, tag="s1")
        nc.scalar.activation(s1[:], s2[:], AF.Sqrt)
        nc.vector.tensor_scalar_max(s1[:], s1[:], 1e-8)
        nc.vector.reciprocal(invb[:], s1[:])

        # inv_s4[p, k] = invb[k, p//32] via transpose-matmul + replication
        invT_ps = mini_pp.tile([BW, 128], FP32, tag="mini")
        nc.tensor.matmul(invT_ps[:], invb[:], ident[:], start=True, stop=True)
        invT_sb = selpool.tile([BW, 128], FP32, tag="invT")
        nc.scalar.activation(invT_sb[:], invT_ps[:], AF.Copy)
        invs4_ps = mini_pp.tile([128, 128], FP32, tag="mini")
        nc.tensor.matmul(invs4_ps[:], rep4[:], invT_sb[:], start=True,
                         stop=True)
        invs4_sb = selpool.tile([128, 128], FP32, tag="invs4")
        nc.scalar.activation(invs4_sb[:], invs4_ps[:], AF.Copy)

        # index transpose via PE (indices as exact fp32)
        idx128 = selpool.tile([128, 128], I16, tag="idx128")
        idxf = selpool.tile([128, 128], FP32, tag="idxf")
        nc.vector.tensor_copy(idxf[:], i16pad[:])
        idxt_ps = mini_pp.tile([128, 128], FP32, tag="mini")
        nc.tensor.matmul(idxt_ps[:], idxf[:], ident[:], start=True, stop=True)
        nc.scalar.activation(idx128[:], idxt_ps[:], AF.Copy)

        # gather Qg[p, k*16+l] = Q4[p, idx[k,l]]
        qg = gpool.tile([128, K * L], FP32, tag="qg")
        nc.gpsimd.ap_gather(
            qg[:].unsqueeze(2), q4_sb[:].unsqueeze(2), idx128[:],
            channels=128, num_elems=K, d=1, num_idxs=K * L)

        # A1 = (Qg - Q4_rep) * inv_rep ; h1 = relu(A1 + b1) -> bf16
        q4rep = q4_sb[:].unsqueeze(2).broadcast_to([128, K, L])
        invrep = invs4_sb[:].unsqueeze(2).broadcast_to([128, K, L])
        a1 = gpool.tile([128, K, L], FP32, tag="a1")
        nc.vector.tensor_sub(a1[:], qg[:].rearrange("p (k l) -> p k l", l=L),
                             q4rep)
        nc.vector.tensor_mul(a1[:], a1[:], invrep)
        h1b = hpool.tile([128, K * L], BF16, tag="h1b")
        nc.scalar.activation(h1b[:].rearrange("p (k l) -> p k l", l=L), a1[:],
                             AF.Relu, bias=b1col[:])

        # L2 + relu2 + L3 + reduce over neighbors
        m3 = opool.tile([128, BW * 128], FP32, tag="m3")
        for p in range(BW // 2):
            a2_ps = a2_pp.tile([128, 2048], FP32)
            for i in range(2):
                a = 2 * p + i
                for h in range(4):
                    nc.tensor.matmul(
                        a2_ps[64 * i:64 * (i + 1), 512 * h:512 * (h + 1)],
                        w2_sb[32 * a:32 * (a + 1), :],
                        h1b[32 * a:32 * (a + 1), 512 * h:512 * (h + 1)],
                        start=True, stop=True,
                        tile_position=(32 * a, 64 * i))
            h2b = hpool.tile([128, 2048], BF16, tag="h2b")
            nc.scalar.activation(h2b[:], a2_ps[:], AF.Relu, bias=b2col[:])
            for i in range(2):
                a = 2 * p + i
                for h in range(4):
                    a3_ps = a3_pp.tile([128, 512], FP32)
                    nc.tensor.matmul(
                        a3_ps[:], w3_sb[64 * i:64 * (i + 1), :],
                        h2b[64 * i:64 * (i + 1), 512 * h:512 * (h + 1)],
                        start=True, stop=True, tile_position=(64 * i, 0))
                    nc.vector.reduce_max(
                        m3[:, 128 * a + 32 * h:128 * a + 32 * (h + 1)],
                        a3_ps[:].rearrange("p (k l) -> p k l", l=L),
                        axis=mybir.AxisListType.X)

        # b3 preload + PE transpose + copy out + DMA
        for a, w in enumerate(wins):
            tr_ps = mini_pp.tile([128, 128], FP32, tag="mini")
            nc.tensor.matmul(tr_ps[:], onesrow[:], b3row[:],
                             start=True, stop=False)
            nc.tensor.matmul(tr_ps[:], m3[:, 128 * a:128 * (a + 1)], ident[:],
                             is_transpose=True, start=False, stop=True)
            out_sb = opool.tile([128, 128], FP32, tag="osb")
            nc.scalar.activation(out_sb[:], tr_ps[:], AF.Copy)
            nc.sync.dma_start(out_d[w], out_sb[:])


_CACHE = {}


def _get_program():
    if "nc" in _CACHE:
        return _CACHE["nc"]
    import concourse.mybir as mybir
    import concourse.tile as tile
    from concourse import bacc

    nc = bacc.Bacc("TRN2", target_bir_lowering=False, debug=False,
                   num_devices=N_CORES)
    specs = {
        "windowsP": ((NWP, 4, 128), mybir.dt.float32),
        "windowsQ": ((NWP, 4, 128), mybir.dt.float32),
        "sqT": ((128, NWP), mybir.dt.float32),
        "w1": ((128, 32), mybir.dt.float32),
        "w2b": ((128, 64), mybir.dt.bfloat16),
        "w3b": ((128, 128), mybir.dt.bfloat16),
        "b1col": ((128, 1), mybir.dt.float32),
        "b2col": ((128, 1), mybir.dt.float32),
        "b3row": ((1, 128), mybir.dt.float32),
        "rep4": ((4, 128), mybir.dt.float32),
        "onesrow": ((1, 128), mybir.dt.float32),
        "ident": ((128, 128), mybir.dt.float32),
    }
    ins = {k: nc.dram_tensor(k, list(shape), dt, kind="ExternalInput").ap()
           for k, (shape, dt) in specs.items()}
    outs = {"out": nc.dram_tensor("out", [NWP, 128, 128], mybir.dt.float32,
                                  kind="ExternalOutput").ap()}
    with tile.TileContext(nc) as tc:
        ctx = ExitStack()
        with ctx:
            _build(ctx, tc, outs, ins, NWP)
    nc.compile()
    _CACHE["nc"] = nc
    return nc


def _make_consts(W1, b1, W2, b2, W3, b3):
    import ml_dtypes
    w1r = np.zeros((128, 32), np.float32)
    for a in range(4):
        w1r[32 * a:32 * a + 3] = W1
    rep4 = np.zeros((4, 128), np.float32)
    for a in range(4):
        rep4[a, 32 * a:32 * a + 32] = 1.0
    return {
        "w1": w1r,
        "w2b": np.tile(np.asarray(W2, np.float32), (4, 1)).astype(
            ml_dtypes.bfloat16),
        "w3b": np.tile(np.asarray(W3, np.float32), (2, 1)).astype(
            ml_dtypes.bfloat16),
        "b1col": np.tile(np.asarray(b1, np.float32), 4)[:, None].copy(),
        "b2col": np.tile(np.asarray(b2, np.float32), 2)[:, None].copy(),
        "b3row": np.asarray(b3, np.float32)[None, :].copy(),
        "rep4": rep4,
        "onesrow": np.ones((1, 128), np.float32),
        "ident": np.eye(128, dtype=np.float32),
    }


def _prep_windows(pts):
    """pts [nw,128,3] -> windowsP/windowsQ [nw,4,128] + sqT [128,nw]."""
    nw = pts.shape[0]
    sq = np.einsum("wkc,wkc->wk", pts, pts)
    winP = np.concatenate([pts, np.ones((nw, K, 1), np.float32)],
                          axis=2).transpose(0, 2, 1)
    winQ = np.concatenate([pts, -0.5 * sq[:, :, None]],
                          axis=2).transpose(0, 2, 1)
    return (np.ascontiguousarray(winP), np.ascontiguousarray(winQ),
            np.ascontiguousarray(sq.T))


def kernel(windows, W1, b1, W2, b2, W3, b3):
    from concourse.bass_utils import run_bass_kernel_spmd

    windows = np.ascontiguousarray(windows, dtype=np.float32)
    consts = _make_consts(W1, b1, W2, b2, W3, b3)
    nc = _get_program()

    out = np.empty((M, 128, 128), np.float32)
    n_launch = M_SHARD // NWP
    for j in range(n_launch):
        in_maps = []
        for c in range(N_CORES):
            lo = c * M_SHARD + j * NWP
            winP, winQ, sqT = _prep_windows(windows[lo:lo + NWP])
            in_maps.append({"windowsP": winP, "windowsQ": winQ, "sqT": sqT,
                            **consts})
        last_err = None
        for _attempt in range(3):
            try:
                res = run_bass_kernel_spmd(nc, in_maps,
                                           core_ids=list(range(N_CORES)))
                break
            except Exception as e:  # retry transient device failures
                last_err = e
        else:
            raise last_err
        for c in range(N_CORES):
            lo = c * M_SHARD + j * NWP
            out[lo:lo + NWP] = res.results[c]["out"]
    return out


if __name__ == "__main__":
    rng = np.random.default_rng(0)
    w = rng.standard_normal((M, 128, 3)).astype(np.float32)

    def _lin(fi, fo):
        lim = 1.0 / np.sqrt(fi)
        return (rng.uniform(-lim, lim, (fi, fo)).astype(np.float32),
                rng.uniform(-lim, lim, fo).astype(np.float32))

    W1, b1 = _lin(3, 32)
    W2, b2 = _lin(32, 64)
    W3, b3 = _lin(64, 128)
    o = kernel(w, W1=W1, b1=b1, W2=W2, b2=b2, W3=W3, b3=b3)
    print(o.shape, o.dtype, float(np.abs(o).max()))
